# revision 1
# baseline (speedup 1.0000x reference)
"""Trainium2 Bass kernel for a single causal attention head.

Reference computation (per batch element b):
    q = x[b] @ Wq; k = x[b] @ Wk; v = x[b] @ Wv          # [T, HD]
    S = q @ k.T;  S = where(tril, S, -inf) / sqrt(C)
    out[b] = softmax(S, -1) @ v                           # [T, HD]

Sharding: pure data parallel — core i computes batch element i (B == 8 ==
n_cores). No collectives.

Device algorithm (per core), designed to avoid all large transposes:
  * host pre-transposes x[b] -> xT [C, T] so the contraction dim (C) lies on
    SBUF partitions with unit-stride DMA.
  * projections: stationary [Wq_i | Wk_i] (M=128) against moving xT pieces
    gives qT/kT stacked in PSUM; Wv alone gives vT; vT is transposed on
    TensorE (small: 64x128 tiles) into natural v layout with a ones column
    appended (v65).
  * scores are computed TRANSPOSED: S_T[s, t] = kT_slice.T @ qT, so that
    exp(S_T) (elementwise, ScalarE, with 1/sqrt(C) folded into the
    activation scale) is directly the moving operand of the second matmul:
    out_unnorm.T[d, t] (+ row-sum in row 64 via the v65 ones column)
    = v65.T @ exp(S_T).  Causal masking = zeroing exp tiles via
    gpsimd.affine_select (diagonal blocks only; fully-masked blocks are
    simply never computed).
  * normalize: transpose the small [65, 128] output tiles back on TensorE,
    multiply by the reciprocal of the row-sum column, DMA out.
"""

import numpy as np

B, T, C, HD = 8, 2048, 1024, 64
NCORES = 8
CHUNK = 512                # t-chunk width for matmul moving operands
NJ = T // CHUNK            # 4 t-chunks
NCT = C // 128             # 8 c-tiles (contraction tiles)
NST = T // 128             # 16 s-tiles (key tiles of 128)
SCALE = 1.0 / np.sqrt(np.float32(C))

# matmul precision mode: "f32" (exact, slow), "f32r" (fp32 replicated, fast),
# "bf16" (fastest DMA+MM, lowest precision)
MODE = "f32"


def build_bass(mode=MODE):
    import concourse.bacc as bacc
    import concourse.tile as tile
    import concourse.mybir as mybir
    from concourse.masks import make_identity

    f32 = mybir.dt.float32
    if mode == "bf16":
        st_dt = mybir.dt.bfloat16     # storage dtype of matmul operands
        mm_cast = None                # matmul reads storage dtype directly
    elif mode == "f32r":
        st_dt = f32
        mm_cast = mybir.dt.float32r   # bitcast matmul operands to f32r
    else:
        st_dt = f32
        mm_cast = None

    def mm(ap):
        return ap.bitcast(mm_cast) if mm_cast is not None else ap

    EXP = mybir.ActivationFunctionType.Exp
    GE = mybir.AluOpType.is_ge

    nc = bacc.Bacc("TRN2", target_bir_lowering=False, debug=False,
                   num_devices=NCORES)
    xt = nc.dram_tensor("xt", [NCT, NJ, 128, CHUNK], st_dt,
                        kind="ExternalInput")
    wqk = nc.dram_tensor("wqk", [128, NCT * 128], st_dt, kind="ExternalInput")
    wv = nc.dram_tensor("wv", [128, NCT * 64], st_dt, kind="ExternalInput")
    out = nc.dram_tensor("out", [T, HD], f32, kind="ExternalOutput")

    with tile.TileContext(nc) as tc:
        with (
            tc.tile_pool(name="consts", bufs=1) as consts,
            tc.tile_pool(name="xin", bufs=NCT * NJ) as xin,
            tc.tile_pool(name="proj", bufs=1) as proj,
            tc.tile_pool(name="es", bufs=4) as es_pool,
            tc.tile_pool(name="small", bufs=4) as small,
            tc.tile_pool(name="psA", bufs=3, space="PSUM") as psA,
            tc.tile_pool(name="psO", bufs=2, space="PSUM") as psO,
            tc.tile_pool(name="psT", bufs=3, space="PSUM") as psT,
        ):
            # ---- constants ----
            ident = consts.tile([128, 128], st_dt, tag="ident")
            make_identity(nc, ident[:])
            ident_f = consts.tile([128, 128], f32, tag="ident_f")
            make_identity(nc, ident_f[:])

            wqk_sb = consts.tile([128, NCT * 128], st_dt, tag="wqk")
            nc.sync.dma_start(wqk_sb[:], wqk[:, :])
            wv_sb = consts.tile([128, NCT * 64], st_dt, tag="wv")
            nc.sync.dma_start(wv_sb[:], wv[:, :])

            # projections live for the whole kernel
            q_sb = proj.tile([64, T], st_dt, tag="q")
            k_sb = proj.tile([64, T], st_dt, tag="k")
            vt_sb = proj.tile([64, T], st_dt, tag="vt")
            v65 = proj.tile([128, NST * 65], st_dt, tag="v65")
            for st in range(NST):
                nc.gpsimd.memset(v65[:, st * 65 + 64: st * 65 + 65], 1.0)

            # ---- input DMA (j-major so chunk 0 lands first) ----
            xts = {}
            for j in range(NJ):
                for i in range(NCT):
                    xtile = xin.tile([128, CHUNK], st_dt, tag="x")
                    nc.sync.dma_start(xtile[:], xt[i, j, :, :])
                    xts[i, j] = xtile

            for j in range(NJ):
                tsl = slice(j * CHUNK, (j + 1) * CHUNK)
                # ---- projections for chunk j ----
                ps_qk = psA.tile([128, CHUNK], f32, tag="mm")
                for i in range(NCT):
                    nc.tensor.matmul(
                        ps_qk[:],
                        mm(wqk_sb[:, i * 128:(i + 1) * 128]),
                        mm(xts[i, j][:]),
                        start=(i == 0), stop=(i == NCT - 1),
                    )
                nc.vector.tensor_copy(q_sb[:, tsl], ps_qk[0:64, :])
                # cross-partition copy (64..128 -> 0..64)
                nc.vector.tensor_copy(k_sb[:, tsl], ps_qk[64:128, :])

                ps_v = psA.tile([64, CHUNK], f32, tag="mm")
                for i in range(NCT):
                    nc.tensor.matmul(
                        ps_v[:],
                        mm(wv_sb[:, i * 64:(i + 1) * 64]),
                        mm(xts[i, j][:]),
                        start=(i == 0), stop=(i == NCT - 1),
                    )
                nc.vector.tensor_copy(vt_sb[:, tsl], ps_v[:, :])

                # v natural layout (+ ones col built at setup)
                for st in range(4 * j, 4 * j + 4):
                    ps_t = psT.tile([128, 64], f32, tag="tp")
                    nc.tensor.transpose(
                        ps_t[:],
                        vt_sb[:, st * 128:(st + 1) * 128],
                        ident[0:64, 0:64],
                    )
                    nc.vector.tensor_copy(v65[:, st * 65: st * 65 + 64],
                                          ps_t[:, :])

                # ---- attention for t-chunk j ----
                ps_oT = psO.tile([128, CHUNK], f32, tag="oT")
                nst = 4 * (j + 1)
                for st in range(nst):
                    ps_s = psA.tile([128, CHUNK], f32, tag="mm")
                    nc.tensor.matmul(
                        ps_s[:],
                        mm(k_sb[:, st * 128:(st + 1) * 128]),
                        mm(q_sb[:, tsl]),
                        start=True, stop=True,
                    )
                    es = es_pool.tile([128, CHUNK], st_dt, tag="es")
                    nc.scalar.activation(es[:], ps_s[:], EXP, scale=float(SCALE))
                    r = st - 4 * j
                    if r >= 0:
                        # keep es[s, t] where t >= s + 128*r, else 0
                        nc.gpsimd.affine_select(
                            out=es[:], in_=es[:], compare_op=GE, fill=0.0,
                            base=-128 * r, channel_multiplier=-1,
                            pattern=[[1, CHUNK]],
                        )
                    nc.tensor.matmul(
                        ps_oT[0:65, :],
                        mm(v65[:, st * 65:(st + 1) * 65]),
                        mm(es[:]),
                        start=(st == 0), stop=(st == nst - 1),
                    )

                oT_sb = small.tile([65, CHUNK], f32, tag="oT_sb")
                nc.vector.tensor_copy(oT_sb[:, :], ps_oT[0:65, :])
                for kk in range(4):
                    ps_o = psT.tile([128, 65], f32, tag="tp")
                    nc.tensor.transpose(
                        ps_o[:],
                        oT_sb[:, kk * 128:(kk + 1) * 128],
                        ident_f[0:65, 0:65],
                    )
                    rec = small.tile([128, 1], f32, tag="rec")
                    nc.vector.reciprocal(rec[:], ps_o[:, 64:65])
                    ob = small.tile([128, 64], f32, tag="ob")
                    nc.vector.tensor_scalar_mul(ob[:], ps_o[:, 0:64], rec[:])
                    tb = 4 * j + kk
                    nc.sync.dma_start(out[tb * 128:(tb + 1) * 128, :], ob[:])

    nc.compile()
    return nc


def prep_inputs(x, Wq, Wk, Wv, mode=MODE):
    """Host-side shard + layout prep. Returns in_maps for 8 cores."""
    if mode == "bf16":
        import ml_dtypes
        cast = lambda a: np.ascontiguousarray(a).astype(ml_dtypes.bfloat16)
    else:
        cast = lambda a: np.ascontiguousarray(a, dtype=np.float32)

    # [Wq_i | Wk_i] interleaved: [128, NCT*128]
    wq_r = Wq.reshape(NCT, 128, HD)
    wk_r = Wk.reshape(NCT, 128, HD)
    wqk = np.concatenate([wq_r, wk_r], axis=2)          # [NCT, 128, 128]
    wqk = wqk.transpose(1, 0, 2).reshape(128, NCT * 128)
    wv = Wv.reshape(NCT, 128, HD).transpose(1, 0, 2).reshape(128, NCT * HD)
    wqk = cast(wqk)
    wv = cast(wv)

    in_maps = []
    for b in range(NCORES):
        xtb = x[b].T                                     # [C, T]
        xtb = xtb.reshape(NCT, 128, NJ, CHUNK).transpose(0, 2, 1, 3)
        in_maps.append({"xt": cast(xtb), "wqk": wqk, "wv": wv})
    return in_maps


_NC_CACHE = {}


def kernel(x, Wq, Wk, Wv):
    from concourse.bass_utils import run_bass_kernel_spmd

    if MODE not in _NC_CACHE:
        _NC_CACHE[MODE] = build_bass(MODE)
    nc = _NC_CACHE[MODE]
    in_maps = prep_inputs(np.asarray(x), np.asarray(Wq), np.asarray(Wk),
                          np.asarray(Wv), MODE)
    res = run_bass_kernel_spmd(nc, in_maps, core_ids=list(range(NCORES)))
    return np.stack([res.results[b]["out"] for b in range(NCORES)], axis=0)
